# revision 37
# baseline (speedup 1.0000x reference)
"""Trainium2 Bass kernel: causal multi-head attention block (QKV proj + RoPE +
causal softmax attention + out proj), distributed over 8 NeuronCores.

Sharding: core = (batch b in 0..3, head-group g in 0..1). Each core computes
the full attention pipeline for its batch and its 16 heads, producing a
partial [T, C] output (its heads' contribution through the out projection).
Host sums the two partials per batch and adds b_proj. No collectives.

Per-core layouts (host-prepared, mostly bf16):
  xt   [C, T]            x[b].T
  wq   [HP, 128, KC*128] W_q columns for group g, RoPE-planar-permuted, tiled
  wk   [HP, 128, KC*128] same for K
  wv   [KC, 128, CL]     W_v columns for group g (natural order)
  wo   [DC, 128, C]      W_proj rows for group g
  cosf/sinf [128, T]     RoPE tables in head-pair row layout (sign folded)
  tri  [128, 128]        causal mask for diagonal blocks (kk <= qq)
  onesw [2, 128]         f32 block-ones used to broadcast softmax reciprocals
  out  [T, C] f32        partial output

On-device per head-pair hp (2 heads stacked on 128 partitions):
  qT/kT [128 dims, T] via matmul with W stationary; RoPE applied on DVE during
  PSUM eviction (planar layout makes the pair-rotation a 32-partition-block
  swap). Scores computed transposed ST[k, q] with 2-head row-tiled K=64
  matmuls; exp on ACT (no max subtraction -- scores are N(0,1)); denominator
  via a ones column appended to V (M=65 matmuls); per-q-block normalization
  with DVE reciprocal + K=2 broadcast matmul.
"""

import numpy as np
import ml_dtypes

import concourse.bass as bass
import concourse.bacc as bacc
import concourse.mybir as mybir
import concourse.tile as tile

BF16 = mybir.dt.bfloat16
F32 = mybir.dt.float32
F32R = mybir.dt.float32r
AF = mybir.ActivationFunctionType
NPBF16 = ml_dtypes.bfloat16

N_EMBD = 2048
N_HEAD = 32
HEAD_DIM = 64
B_FULL = 4
T_FULL = 2048
N_CORES = 8
HLOC_FULL = 16  # heads per core


def build_graph(T=2048, C=2048, HLOC=16, qk_bias=False, v_bias=False):
    D = HEAD_DIM
    HP = HLOC // 2          # head pairs per core
    CL = HLOC * D           # local head dims
    KC = C // 128           # contraction chunks for projections
    TT = T // 128           # token tiles
    QBS = min(512, T)       # q-block size
    NQB = T // QBS
    TSW = min(512, T)       # token slice width for qkv psum
    NTS = T // TSW
    VN = min(512, CL)       # v matmul free width
    NVS = CL // VN
    DC = CL // 128          # out-proj contraction chunks
    CS = min(512, C)        # out-proj col slice
    NCS = C // CS
    SCALE = 1.0 / float(np.sqrt(D))

    nc = bacc.Bacc(None, target_bir_lowering=False, debug=False)

    xt_d = nc.declare_dram_parameter("xt", [C, T], BF16, False)
    wq_d = nc.declare_dram_parameter("wq", [HP, 128, KC * 128], BF16, False)
    wk_d = nc.declare_dram_parameter("wk", [HP, 128, KC * 128], BF16, False)
    wv_d = nc.declare_dram_parameter("wv", [KC, 128, CL], BF16, False)
    wo_d = nc.declare_dram_parameter("wo", [DC, 128, C], BF16, False)
    cos_d = nc.declare_dram_parameter("cosf", [128, T], BF16, False)
    sin_d = nc.declare_dram_parameter("sinf", [128, T], BF16, False)
    tri_d = nc.declare_dram_parameter("tri", [128, 128], BF16, False)
    onw_d = nc.declare_dram_parameter("onesw", [2, 128], BF16, False)
    if qk_bias:
        bqk_d = nc.declare_dram_parameter("bqk", [128, 2 * HP], F32, False)
    if v_bias:
        bv_d = nc.declare_dram_parameter("bv", [128, HP], F32, False)
    out_d = nc.declare_dram_parameter("out", [T, C], F32, True)

    with tile.TileContext(nc) as tc:
        with (
            tc.tile_pool(name="const", bufs=1) as constp,
            tc.tile_pool(name="xt", bufs=1) as xtp,
            tc.tile_pool(name="vall", bufs=1) as vallp,
            tc.tile_pool(name="yt", bufs=1) as ytp,
            tc.tile_pool(name="psmm", bufs=2, space="PSUM") as psmm,
            tc.tile_pool(name="pssc", bufs=2, space="PSUM") as pssc,
            tc.tile_pool(name="psyt", bufs=1, space="PSUM") as psyt,
        ):
            # ---- constants ----
            cosf = constp.tile([128, T], BF16, name="cosf", tag="cosf")
            sinf = constp.tile([128, T], BF16, name="sinf", tag="sinf")
            tri = constp.tile([128, 128], BF16, name="tri", tag="tri")
            onw = constp.tile([2, 128], BF16, name="onw", tag="onw")
            nc.sync.dma_start(cosf[:], cos_d.ap())
            nc.sync.dma_start(sinf[:], sin_d.ap())
            nc.sync.dma_start(tri[:], tri_d.ap())
            nc.sync.dma_start(onw[:], onw_d.ap())
            if qk_bias:
                bqk = constp.tile([128, 2 * HP], F32, name="bqk", tag="bqk")
                nc.sync.dma_start(bqk[:], bqk_d.ap())
            if v_bias:
                bv = constp.tile([128, HP], F32, name="bv", tag="bv")
                nc.sync.dma_start(bv[:], bv_d.ap())

            # ---- x^T resident ----
            xt = []
            for k in range(KC):
                xk = xtp.tile([128, T], BF16, name=f"xt{k}", tag=f"xt{k}")
                nc.sync.dma_start(xk[:], xt_d.ap()[k * 128:(k + 1) * 128, :])
                xt.append(xk)

            # ---- v_all tiles (65-packed: 64 dims + ones column per head) ----
            vall = []
            for t in range(TT):
                vt = vallp.tile([128, HLOC * 65], BF16, name=f"vall{t}",
                                tag=f"vall{t}")
                vall.append(vt)

            # ---- yT accumulator tiles ----
            ytall = []
            for d_ in range(DC):
                yt_ = ytp.tile([128, T], BF16, name=f"yt{d_}", tag=f"yt{d_}")
                ytall.append(yt_)

            # ================= phase 1: V projection =================
            with tc.tile_pool(name="wv", bufs=1) as wvp:
                wv = []
                for k in range(KC):
                    wvk = wvp.tile([128, CL], BF16, name=f"wv{k}", tag=f"wv{k}")
                    nc.sync.dma_start(wvk[:], wv_d.ap()[k])
                    wv.append(wvk)
                for t in range(TT):
                    v3 = vall[t][:].rearrange("p (h c) -> p h c", c=65)
                    nc.vector.memset(v3[:, :, 64:65], 1.0)
                    for ns in range(NVS):
                        pv = psmm.tile([128, VN], F32, name="pv", tag="mm")
                        for k in range(KC):
                            nc.tensor.matmul(
                                pv[:],
                                xt[k][:, t * 128:(t + 1) * 128],
                                wv[k][:, ns * VN:(ns + 1) * VN],
                                start=(k == 0), stop=(k == KC - 1),
                            )
                        nh = VN // 64
                        src = pv[:].rearrange("p (h c) -> p h c", c=64)
                        dst = v3[:, ns * nh:(ns + 1) * nh, 0:64]
                        nc.vector.tensor_copy(dst, src)

            # ================= phase 2: per head-pair =================
            with (
                tc.tile_pool(name="wqk", bufs=2) as wqkp,
                tc.tile_pool(name="qkt", bufs=2) as qktp,
                tc.tile_pool(name="rope", bufs=3) as ropep,
                tc.tile_pool(name="esc", bufs=4) as ep,
                tc.tile_pool(name="norm", bufs=1) as normp,
            ):
                def new_qkv(hp):
                    """Allocate tiles + DMA for head-pair hp; return
                    (qt, kt, step-generator emitting QKV matmuls + rope)."""
                    wq = wqkp.tile([128, KC * 128], BF16, name="wq", tag="wq")
                    wk = wqkp.tile([128, KC * 128], BF16, name="wk", tag="wk")
                    nc.sync.dma_start(wq[:], wq_d.ap()[hp])
                    nc.sync.dma_start(wk[:], wk_d.ap()[hp])
                    qt = qktp.tile([128, T], BF16, name="qt", tag="qt")
                    kt = qktp.tile([128, T], BF16, name="kt", tag="kt")

                    def steps():
                        # ts-major so early token slices of BOTH q and k land
                        # first -- the next head-pair's attention can start
                        # its first q-block without waiting for the whole
                        # K projection
                        for ts in range(NTS):
                            for (wsb, dst, bcol) in ((wq, qt, hp),
                                                     (wk, kt, HP + hp)):
                                sl = slice(ts * TSW, (ts + 1) * TSW)
                                pq = psmm.tile([128, TSW], F32, name="pq",
                                               tag="mm")
                                for k0 in range(0, KC, 4):
                                    for k in range(k0, min(k0 + 4, KC)):
                                        nc.tensor.matmul(
                                            pq[:],
                                            wsb[:, k * 128:(k + 1) * 128],
                                            xt[k][:, sl],
                                            start=(k == 0),
                                            stop=(k == KC - 1),
                                        )
                                    yield
                                raw = ropep.tile([128, TSW], BF16, name="raw",
                                                 tag="raw")
                                nc.vector.tensor_copy(raw[:], pq[:])
                                if qk_bias:
                                    nc.vector.tensor_scalar_add(
                                        raw[:], raw[:], bqk[:, bcol:bcol + 1])
                                t1 = ropep.tile([128, TSW], BF16, name="t1",
                                                tag="t1")
                                nc.vector.tensor_mul(t1[:], raw[:], cosf[:, sl])
                                # sinf rows are host-swapped (row r holds the
                                # sin for destination row r^32) so both inputs
                                # read at the same base partition.
                                t2 = ropep.tile([128, TSW], BF16, name="t2",
                                                tag="t2")
                                for blk in range(4):
                                    sb_ = blk ^ 1
                                    nc.vector.tensor_mul(
                                        t2[blk * 32:(blk + 1) * 32, :],
                                        raw[sb_ * 32:(sb_ + 1) * 32, :],
                                        sinf[sb_ * 32:(sb_ + 1) * 32, sl],
                                    )
                                nc.vector.tensor_add(dst[:, sl], t1[:], t2[:])
                                yield

                    return qt, kt, steps()

                cur = new_qkv(0)
                for _ in cur[2]:
                    pass

                for hp in range(HP):
                    qt, kt = cur[0], cur[1]
                    bg = None
                    nxt = None
                    if hp + 1 < HP:
                        nxt = new_qkv(hp + 1)
                        bg = nxt[2]

                    def score_group(kt_i, nfull, q0):
                        if kt_i < nfull:
                            off, N = 0, QBS
                        else:
                            i = kt_i - nfull
                            off, N = 128 * i, QBS - 128 * i
                        ksl = slice(kt_i * 128, (kt_i + 1) * 128)
                        qsl = slice(q0 + off, q0 + QBS)
                        # both heads' scores in one 2-bank psum tensor (the
                        # halves are bank-aligned) so one ACT exp covers both
                        sc2 = pssc.tile([128, 2 * QBS], F32, name="sc2",
                                        tag="sc")
                        nc.tensor.matmul(sc2[:, 0:N], kt[0:64, ksl],
                                         qt[0:64, qsl], start=True, stop=True)
                        nc.tensor.matmul(sc2[:, QBS:QBS + N], kt[64:128, ksl],
                                         qt[64:128, qsl], start=True,
                                         stop=True)
                        e2 = ep.tile([128, 2 * QBS], BF16, name="e2", tag="e")
                        if N == QBS:
                            nc.scalar.activation(e2[:, 0:2 * QBS],
                                                 sc2[:, 0:2 * QBS],
                                                 AF.Exp, scale=SCALE)
                        else:
                            nc.scalar.activation(e2[:, 0:N], sc2[:, 0:N],
                                                 AF.Exp, scale=SCALE)
                            nc.scalar.activation(e2[:, QBS:QBS + N],
                                                 sc2[:, QBS:QBS + N],
                                                 AF.Exp, scale=SCALE)
                        if kt_i >= nfull:
                            # masks on idle GPSIMD: keeps the exp->yT chain
                            # off the DVE queue (rope/norm contention)
                            nc.gpsimd.tensor_mul(e2[:, 0:128], e2[:, 0:128],
                                                 tri[:])
                            nc.gpsimd.tensor_mul(e2[:, QBS:QBS + 128],
                                                 e2[:, QBS:QBS + 128], tri[:])
                        return (kt_i, off, N, e2)

                    def yt_group(g, pyA, pyB, nkt):
                        kt_i, off, N, e2 = g
                        vA = vall[kt_i][:, (2 * hp) * 65:(2 * hp) * 65 + 65]
                        vB = vall[kt_i][:, (2 * hp + 1) * 65:
                                        (2 * hp + 1) * 65 + 65]
                        nc.tensor.matmul(pyA[:, off:QBS], vA, e2[:, 0:N],
                                         start=(kt_i == 0),
                                         stop=(kt_i == nkt - 1))
                        nc.tensor.matmul(pyB[:, off:QBS], vB,
                                         e2[:, QBS:QBS + N],
                                         start=(kt_i == 0),
                                         stop=(kt_i == nkt - 1))

                    # ---- attention for this head pair ----
                    for qb in range(NQB):
                        q0 = qb * QBS
                        pyA = psyt.tile([65, QBS], F32, name="pyA", tag="ytA")
                        pyB = psyt.tile([65, QBS], F32, name="pyB", tag="ytB")
                        nfull = q0 // 128
                        ndiag = QBS // 128
                        nkt = nfull + ndiag
                        pend = None
                        for kt_i in range(nkt):
                            g = score_group(kt_i, nfull, q0)
                            # background QKV work lands between the score and
                            # the exp-dependent yT so the exp latency is
                            # hidden without blocking the in-order PE stream
                            if bg is not None:
                                next(bg, None)
                            if pend is not None:
                                yt_group(pend, pyA, pyB, nkt)
                            pend = g
                        yt_group(pend, pyA, pyB, nkt)

                        # release the psum accumulators to SBUF immediately,
                        # normalize from the SBUF copies
                        yAsb = normp.tile([65, QBS], F32, name="yAsb",
                                          tag="yAsb")
                        yBsb = normp.tile([65, QBS], F32, name="yBsb",
                                          tag="yBsb")
                        nc.vector.tensor_copy(yAsb[:], pyA[:])
                        nc.vector.tensor_copy(yBsb[:], pyB[:])
                        r2s = normp.tile([1, 2 * QBS], F32, name="r2s",
                                         tag="r2s")
                        nc.gpsimd.tensor_copy(r2s[0:1, 0:QBS], yAsb[64:65, :])
                        nc.gpsimd.tensor_copy(r2s[0:1, QBS:2 * QBS],
                                              yBsb[64:65, :])
                        r2f = normp.tile([1, 2 * QBS], F32, name="r2f",
                                         tag="r2f")
                        nc.vector.reciprocal_approx_fast(r2f[:], r2s[:])
                        # broadcast the reciprocal rows to 128 partitions on
                        # the otherwise-idle GPSIMD engine (replaces two K=1
                        # matmuls + three DVE casts)
                        bc2 = normp.tile([128, 2 * QBS], F32, name="bc2",
                                         tag="bc2")
                        nc.gpsimd.partition_broadcast(bc2[:], r2f[:])
                        qbs = slice(q0, q0 + QBS)
                        nc.vector.tensor_mul(ytall[hp][0:64, qbs],
                                             yAsb[0:64, :], bc2[0:64, 0:QBS])
                        nc.vector.tensor_mul(ytall[hp][64:128, qbs],
                                             yBsb[0:64, :],
                                             bc2[0:64, QBS:2 * QBS])
                        if v_bias:
                            nc.vector.tensor_scalar_add(
                                ytall[hp][0:64, qbs], ytall[hp][0:64, qbs],
                                bv[0:64, hp:hp + 1])
                            nc.vector.tensor_scalar_add(
                                ytall[hp][64:128, qbs], ytall[hp][64:128, qbs],
                                bv[64:128, hp:hp + 1])

                    if bg is not None:
                        for _ in bg:
                            pass
                        cur = nxt

            # ================= phase 3: out projection =================
            with (
                tc.tile_pool(name="wo", bufs=1) as wop,
                tc.tile_pool(name="ost", bufs=4) as ostp,
            ):
                wo = []
                for d_ in range(DC):
                    wod = wop.tile([128, C], BF16, name=f"wo{d_}", tag=f"wo{d_}")
                    nc.sync.dma_start(wod[:], wo_d.ap()[d_])
                    wo.append(wod)
                for t in range(TT):
                    for cs in range(NCS):
                        po = psmm.tile([128, CS], F32, name="po", tag="mm")
                        for d_ in range(DC):
                            nc.tensor.matmul(
                                po[:],
                                ytall[d_][:, t * 128:(t + 1) * 128],
                                wo[d_][:, cs * CS:(cs + 1) * CS],
                                start=(d_ == 0), stop=(d_ == DC - 1),
                            )
                        st = ostp.tile([128, CS], F32, name="st", tag="ost")
                        nc.scalar.copy(st[:], po[:])
                        nc.sync.dma_start(
                            out_d.ap()[t * 128:(t + 1) * 128,
                                       cs * CS:(cs + 1) * CS],
                            st[:])

    nc.compile()
    return nc


# ---------------------------------------------------------------------------
# host-side sharding
# ---------------------------------------------------------------------------

def _planar_perm():
    """Within-head column permutation: even dims -> 0..31, odd -> 32..63."""
    p = np.empty(HEAD_DIM, dtype=np.int64)
    p[:32] = 2 * np.arange(32)
    p[32:] = 2 * np.arange(32) + 1
    return p


def _rope_tables(T):
    theta = 1.0 / (10000.0 ** (np.arange(0, HEAD_DIM, 2, dtype=np.float64)
                               / HEAD_DIM))  # [32]
    idx = np.outer(np.arange(T, dtype=np.float64), theta)  # [T, 32]
    cos = np.cos(idx).astype(np.float32)
    sin = np.sin(idx).astype(np.float32)
    cosf = np.empty((128, T), dtype=np.float32)
    sinf = np.empty((128, T), dtype=np.float32)
    for r in range(128):
        i = r % 32
        lo = ((r // 32) % 2 == 0)
        cosf[r] = cos[:, i]
        sinf[r] = (-sin[:, i]) if lo else sin[:, i]
    # device reads the sin table at the *source* rows of the pair swap
    # (row r holds the value destined for row r^32), so swap 32-row blocks
    sinf = sinf.reshape(4, 32, T)[[1, 0, 3, 2]].reshape(128, T)
    return cosf, sinf


def make_in_maps(x, W_qkv, b_qkv, W_proj, T, C, HLOC, qk_bias, v_bias):
    B = x.shape[0]
    D = HEAD_DIM
    HP = HLOC // 2
    CL = HLOC * D
    KC = C // 128
    DC = CL // 128
    NGRP = (C // D) // HLOC  # head groups

    Wq = np.asarray(W_qkv[:, 0:C], dtype=np.float32)
    Wk = np.asarray(W_qkv[:, C:2 * C], dtype=np.float32)
    Wv = np.asarray(W_qkv[:, 2 * C:3 * C], dtype=np.float32)
    bq = np.asarray(b_qkv[0:C], dtype=np.float32)
    bk = np.asarray(b_qkv[C:2 * C], dtype=np.float32)
    bv_ = np.asarray(b_qkv[2 * C:3 * C], dtype=np.float32)

    perm = _planar_perm()
    cosf, sinf = _rope_tables(T)
    tri = (np.arange(128)[:, None] <= np.arange(128)[None, :])
    onesw = np.zeros((2, 128), dtype=NPBF16)
    onesw[0, 0:64] = 1.0
    onesw[1, 64:128] = 1.0

    def to_bf(a):
        return np.ascontiguousarray(a.astype(NPBF16))

    grp = {}
    for g in range(NGRP):
        cols_qk = np.concatenate(
            [(g * HLOC + h) * D + perm for h in range(HLOC)])
        cols_v = np.concatenate(
            [(g * HLOC + h) * D + np.arange(D) for h in range(HLOC)])
        wq_g = Wq[:, cols_qk]   # [C, CL]
        wk_g = Wk[:, cols_qk]
        wv_g = Wv[:, cols_v]
        wo_g = np.asarray(W_proj[g * CL:(g + 1) * CL, :], dtype=np.float32)

        ent = {
            "wq": to_bf(wq_g.reshape(KC, 128, HP, 128)
                        .transpose(2, 1, 0, 3).reshape(HP, 128, KC * 128)),
            "wk": to_bf(wk_g.reshape(KC, 128, HP, 128)
                        .transpose(2, 1, 0, 3).reshape(HP, 128, KC * 128)),
            "wv": to_bf(wv_g.reshape(KC, 128, CL)),
            "wo": to_bf(wo_g.reshape(DC, 128, C)),
            "cosf": to_bf(cosf),
            "sinf": to_bf(sinf),
            "tri": to_bf(tri.astype(np.float32)),
            "onesw": onesw,
        }
        if qk_bias:
            bqk_t = np.empty((128, 2 * HP), dtype=np.float32)
            bq_g = bq[cols_qk]
            bk_g = bk[cols_qk]
            for hp in range(HP):
                bqk_t[:, hp] = bq_g[hp * 128:(hp + 1) * 128]
                bqk_t[:, HP + hp] = bk_g[hp * 128:(hp + 1) * 128]
            ent["bqk"] = bqk_t
        if v_bias:
            bv_t = np.empty((128, HP), dtype=np.float32)
            bv_g = bv_[cols_v]
            for hp in range(HP):
                bv_t[:, hp] = bv_g[hp * 128:(hp + 1) * 128]
            ent["bv"] = bv_t
        grp[g] = ent

    in_maps = []
    for core in range(B * NGRP):
        b, g = core // NGRP, core % NGRP
        m = dict(grp[g])
        m["xt"] = to_bf(np.asarray(x[b], dtype=np.float32).T)
        in_maps.append(m)
    return in_maps


_CACHE = {}


def _get_graph(T, C, HLOC, qk_bias, v_bias):
    key = (T, C, HLOC, qk_bias, v_bias)
    if key not in _CACHE:
        _CACHE[key] = build_graph(T, C, HLOC, qk_bias, v_bias)
    return _CACHE[key]


def _ensure_ntff_hook():
    """Register the axon NTFF profile hook if the image's antenv lacks it."""
    import sys
    import types
    import antenv
    try:
        from antenv import axon_hooks  # noqa: F401
    except ImportError:
        mod = types.ModuleType("antenv.axon_hooks")
        mod._hook = None

        def set_axon_ntff_profile_hook(h, _m=mod):
            _m._hook = h

        def get_axon_ntff_profile_hook(_m=mod):
            return _m._hook

        mod.set_axon_ntff_profile_hook = set_axon_ntff_profile_hook
        mod.get_axon_ntff_profile_hook = get_axon_ntff_profile_hook
        sys.modules["antenv.axon_hooks"] = mod
        antenv.axon_hooks = mod
    from antenv.axon_hooks import (get_axon_ntff_profile_hook,
                                   set_axon_ntff_profile_hook)
    if get_axon_ntff_profile_hook() is None:
        from trn_agent_boot.trn_boot import _ntff_profile_via_ctypes
        set_axon_ntff_profile_hook(
            _ntff_profile_via_ctypes("/opt/axon/libaxon_pjrt.so"))


def run(inputs, trace=False):
    from concourse.bass_utils import run_bass_kernel_spmd
    if trace:
        try:
            _ensure_ntff_hook()
        except Exception as e:
            print(f"ntff hook setup failed: {e}")
    x = np.asarray(inputs["x"])
    W_qkv = np.asarray(inputs["W_qkv"])
    b_qkv = np.asarray(inputs["b_qkv"])
    W_proj = np.asarray(inputs["W_proj"])
    b_proj = np.asarray(inputs["b_proj"])
    B, T, C = x.shape
    HLOC = HLOC_FULL
    NGRP = (C // HEAD_DIM) // HLOC

    qk_bias = bool(np.any(b_qkv[0:2 * C]))
    v_bias = bool(np.any(b_qkv[2 * C:]))
    nc = _get_graph(T, C, HLOC, qk_bias, v_bias)
    in_maps = make_in_maps(x, W_qkv, b_qkv, W_proj, T, C, HLOC,
                           qk_bias, v_bias)
    res = run_bass_kernel_spmd(nc, in_maps, core_ids=list(range(len(in_maps))),
                               trace=trace)
    out = np.empty((B, T, C), dtype=np.float32)
    for b in range(B):
        acc = None
        for g in range(NGRP):
            part = res.results[b * NGRP + g]["out"]
            acc = part if acc is None else acc + part
        out[b] = acc + b_proj[None, :].astype(np.float32)
    return out, res


def kernel(**inputs):
    out, _ = run(inputs, trace=False)
    return out


# revision 38
# speedup vs baseline: 1.5777x; 1.5777x over previous
"""Trainium2 Bass kernel: causal multi-head attention block (QKV proj + RoPE +
causal softmax attention + out proj), distributed over 8 NeuronCores.

Sharding: core = (batch b in 0..3, head-group g in 0..1). Each core computes
the full attention pipeline for its batch and its 16 heads, producing a
partial [T, C] output (its heads' contribution through the out projection).
Host sums the two partials per batch and adds b_proj. No collectives.

Per-core layouts (host-prepared, mostly bf16):
  xt   [C, T]            x[b].T
  wq   [HP, 128, KC*128] W_q columns for group g, RoPE-planar-permuted, tiled
  wk   [HP, 128, KC*128] same for K
  wv   [KC, 128, CL]     W_v columns for group g (natural order)
  wo   [DC, 128, C]      W_proj rows for group g
  cosf/sinf [128, T]     RoPE tables in head-pair row layout (sign folded)
  tri  [128, 128]        causal mask for diagonal blocks (kk <= qq)
  onesw [2, 128]         f32 block-ones used to broadcast softmax reciprocals
  out  [T, C] f32        partial output

On-device per head-pair hp (2 heads stacked on 128 partitions):
  qT/kT [128 dims, T] via matmul with W stationary; RoPE applied on DVE during
  PSUM eviction (planar layout makes the pair-rotation a 32-partition-block
  swap). Scores computed transposed ST[k, q] with 2-head row-tiled K=64
  matmuls; exp on ACT (no max subtraction -- scores are N(0,1)); denominator
  via a ones column appended to V (M=65 matmuls); per-q-block normalization
  with DVE reciprocal + K=2 broadcast matmul.
"""

import numpy as np
import ml_dtypes

import concourse.bass as bass
import concourse.bacc as bacc
import concourse.mybir as mybir
import concourse.tile as tile

BF16 = mybir.dt.bfloat16
F32 = mybir.dt.float32
F32R = mybir.dt.float32r
AF = mybir.ActivationFunctionType
NPBF16 = ml_dtypes.bfloat16

N_EMBD = 2048
N_HEAD = 32
HEAD_DIM = 64
B_FULL = 4
T_FULL = 2048
N_CORES = 8
HLOC_FULL = 16  # heads per core


def build_graph(T=2048, C=2048, HLOC=16, qk_bias=False, v_bias=False):
    D = HEAD_DIM
    HP = HLOC // 2          # head pairs per core
    CL = HLOC * D           # local head dims
    KC = C // 128           # contraction chunks for projections
    TT = T // 128           # token tiles
    QBS = min(512, T)       # q-block size
    NQB = T // QBS
    TSW = min(512, T)       # token slice width for qkv psum
    NTS = T // TSW
    VN = min(512, CL)       # v matmul free width
    NVS = CL // VN
    DC = CL // 128          # out-proj contraction chunks
    CS = min(512, C)        # out-proj col slice
    NCS = C // CS
    SCALE = 1.0 / float(np.sqrt(D))

    nc = bacc.Bacc(None, target_bir_lowering=False, debug=False)

    xt_d = nc.declare_dram_parameter("xt", [C, T], BF16, False)
    wq_d = nc.declare_dram_parameter("wq", [HP, 128, KC * 128], BF16, False)
    wk_d = nc.declare_dram_parameter("wk", [HP, 128, KC * 128], BF16, False)
    wv_d = nc.declare_dram_parameter("wv", [KC, 128, CL], BF16, False)
    wo_d = nc.declare_dram_parameter("wo", [DC, 128, C], BF16, False)
    cos_d = nc.declare_dram_parameter("cosf", [128, T], BF16, False)
    sin_d = nc.declare_dram_parameter("sinf", [128, T], BF16, False)
    tri_d = nc.declare_dram_parameter("tri", [128, 128], BF16, False)
    onw_d = nc.declare_dram_parameter("onesw", [2, 128], BF16, False)
    if qk_bias:
        bqk_d = nc.declare_dram_parameter("bqk", [128, 2 * HP], F32, False)
    if v_bias:
        bv_d = nc.declare_dram_parameter("bv", [128, HP], F32, False)
    out_d = nc.declare_dram_parameter("out", [T, C], F32, True)

    with tile.TileContext(nc) as tc:
        with (
            tc.tile_pool(name="const", bufs=1) as constp,
            tc.tile_pool(name="xt", bufs=1) as xtp,
            tc.tile_pool(name="vall", bufs=1) as vallp,
            tc.tile_pool(name="yt", bufs=1) as ytp,
            tc.tile_pool(name="psmm", bufs=2, space="PSUM") as psmm,
            tc.tile_pool(name="pssc", bufs=2, space="PSUM") as pssc,
            tc.tile_pool(name="psyt", bufs=1, space="PSUM") as psyt,
        ):
            # ---- constants ----
            cosf = constp.tile([128, T], BF16, name="cosf", tag="cosf")
            sinf = constp.tile([128, T], BF16, name="sinf", tag="sinf")
            tri = constp.tile([128, 128], BF16, name="tri", tag="tri")
            onw = constp.tile([2, 128], BF16, name="onw", tag="onw")
            nc.sync.dma_start(cosf[:], cos_d.ap())
            nc.sync.dma_start(sinf[:], sin_d.ap())
            nc.sync.dma_start(tri[:], tri_d.ap())
            nc.sync.dma_start(onw[:], onw_d.ap())
            if qk_bias:
                bqk = constp.tile([128, 2 * HP], F32, name="bqk", tag="bqk")
                nc.sync.dma_start(bqk[:], bqk_d.ap())
            if v_bias:
                bv = constp.tile([128, HP], F32, name="bv", tag="bv")
                nc.sync.dma_start(bv[:], bv_d.ap())

            # ---- x^T resident ----
            xt = []
            for k in range(KC):
                xk = xtp.tile([128, T], BF16, name=f"xt{k}", tag=f"xt{k}")
                nc.sync.dma_start(xk[:], xt_d.ap()[k * 128:(k + 1) * 128, :])
                xt.append(xk)

            # ---- v_all tiles (65-packed: 64 dims + ones column per head) ----
            vall = []
            for t in range(TT):
                vt = vallp.tile([128, HLOC * 65], BF16, name=f"vall{t}",
                                tag=f"vall{t}")
                vall.append(vt)

            # ---- yT accumulator tiles ----
            ytall = []
            for d_ in range(DC):
                yt_ = ytp.tile([128, T], BF16, name=f"yt{d_}", tag=f"yt{d_}")
                ytall.append(yt_)

            # ================= phase 1: V projection =================
            with tc.tile_pool(name="wv", bufs=1) as wvp:
                wv = []
                for k in range(KC):
                    wvk = wvp.tile([128, CL], BF16, name=f"wv{k}", tag=f"wv{k}")
                    nc.sync.dma_start(wvk[:], wv_d.ap()[k])
                    wv.append(wvk)
                for t in range(TT):
                    v3 = vall[t][:].rearrange("p (h c) -> p h c", c=65)
                    nc.vector.memset(v3[:, :, 64:65], 1.0)
                    for ns in range(NVS):
                        pv = psmm.tile([128, VN], F32, name="pv", tag="mm")
                        for k in range(KC):
                            nc.tensor.matmul(
                                pv[:],
                                xt[k][:, t * 128:(t + 1) * 128],
                                wv[k][:, ns * VN:(ns + 1) * VN],
                                start=(k == 0), stop=(k == KC - 1),
                            )
                        nh = VN // 64
                        src = pv[:].rearrange("p (h c) -> p h c", c=64)
                        dst = v3[:, ns * nh:(ns + 1) * nh, 0:64]
                        nc.vector.tensor_copy(dst, src)

            # ================= phase 2: per head-pair =================
            with (
                tc.tile_pool(name="wqk", bufs=2) as wqkp,
                tc.tile_pool(name="qkt", bufs=2) as qktp,
                tc.tile_pool(name="rope", bufs=3) as ropep,
                tc.tile_pool(name="esc", bufs=4) as ep,
                tc.tile_pool(name="norm", bufs=1) as normp,
            ):
                def new_qkv(hp):
                    """Allocate tiles + DMA for head-pair hp; return
                    (qt, kt, step-generator emitting QKV matmuls + rope)."""
                    wq = wqkp.tile([128, KC * 128], BF16, name="wq", tag="wq")
                    wk = wqkp.tile([128, KC * 128], BF16, name="wk", tag="wk")
                    nc.sync.dma_start(wq[:], wq_d.ap()[hp])
                    nc.sync.dma_start(wk[:], wk_d.ap()[hp])
                    qt = qktp.tile([128, T], BF16, name="qt", tag="qt")
                    kt = qktp.tile([128, T], BF16, name="kt", tag="kt")

                    def steps():
                        # ts-major so early token slices of BOTH q and k land
                        # first -- the next head-pair's attention can start
                        # its first q-block without waiting for the whole
                        # K projection
                        for ts in range(NTS):
                            for (wsb, dst, bcol) in ((wq, qt, hp),
                                                     (wk, kt, HP + hp)):
                                sl = slice(ts * TSW, (ts + 1) * TSW)
                                pq = psmm.tile([128, TSW], F32, name="pq",
                                               tag="mm")
                                for k0 in range(0, KC, 4):
                                    for k in range(k0, min(k0 + 4, KC)):
                                        nc.tensor.matmul(
                                            pq[:],
                                            wsb[:, k * 128:(k + 1) * 128],
                                            xt[k][:, sl],
                                            start=(k == 0),
                                            stop=(k == KC - 1),
                                        )
                                    yield
                                raw = ropep.tile([128, TSW], BF16, name="raw",
                                                 tag="raw")
                                nc.vector.tensor_copy(raw[:], pq[:])
                                if qk_bias:
                                    nc.vector.tensor_scalar_add(
                                        raw[:], raw[:], bqk[:, bcol:bcol + 1])
                                t1 = ropep.tile([128, TSW], BF16, name="t1",
                                                tag="t1")
                                nc.vector.tensor_mul(t1[:], raw[:], cosf[:, sl])
                                # sinf rows are host-swapped (row r holds the
                                # sin for destination row r^32) so both inputs
                                # read at the same base partition.
                                t2 = ropep.tile([128, TSW], BF16, name="t2",
                                                tag="t2")
                                for blk in range(4):
                                    sb_ = blk ^ 1
                                    nc.vector.tensor_mul(
                                        t2[blk * 32:(blk + 1) * 32, :],
                                        raw[sb_ * 32:(sb_ + 1) * 32, :],
                                        sinf[sb_ * 32:(sb_ + 1) * 32, sl],
                                    )
                                nc.vector.tensor_add(dst[:, sl], t1[:], t2[:])
                                yield

                    return qt, kt, steps()

                cur = new_qkv(0)
                for _ in cur[2]:
                    pass

                for hp in range(HP):
                    qt, kt = cur[0], cur[1]
                    bg = None
                    nxt = None
                    if hp + 1 < HP:
                        nxt = new_qkv(hp + 1)
                        bg = nxt[2]

                    def score_group(kt_i, nfull, q0):
                        if kt_i < nfull:
                            off, N = 0, QBS
                        else:
                            i = kt_i - nfull
                            off, N = 128 * i, QBS - 128 * i
                        ksl = slice(kt_i * 128, (kt_i + 1) * 128)
                        qsl = slice(q0 + off, q0 + QBS)
                        # both heads' scores in one 2-bank psum tensor (the
                        # halves are bank-aligned) so one ACT exp covers both
                        sc2 = pssc.tile([128, 2 * QBS], F32, name="sc2",
                                        tag="sc")
                        nc.tensor.matmul(sc2[:, 0:N], kt[0:64, ksl],
                                         qt[0:64, qsl], start=True, stop=True)
                        nc.tensor.matmul(sc2[:, QBS:QBS + N], kt[64:128, ksl],
                                         qt[64:128, qsl], start=True,
                                         stop=True)
                        e2 = ep.tile([128, 2 * QBS], BF16, name="e2", tag="e")
                        if N == QBS:
                            nc.scalar.activation(e2[:, 0:2 * QBS],
                                                 sc2[:, 0:2 * QBS],
                                                 AF.Exp, scale=SCALE)
                        else:
                            nc.scalar.activation(e2[:, 0:N], sc2[:, 0:N],
                                                 AF.Exp, scale=SCALE)
                            nc.scalar.activation(e2[:, QBS:QBS + N],
                                                 sc2[:, QBS:QBS + N],
                                                 AF.Exp, scale=SCALE)
                        if kt_i >= nfull:
                            nc.vector.tensor_mul(e2[:, 0:128], e2[:, 0:128],
                                                 tri[:])
                            nc.vector.tensor_mul(e2[:, QBS:QBS + 128],
                                                 e2[:, QBS:QBS + 128], tri[:])
                        return (kt_i, off, N, e2)

                    def yt_group(g, pyA, pyB, nkt):
                        kt_i, off, N, e2 = g
                        vA = vall[kt_i][:, (2 * hp) * 65:(2 * hp) * 65 + 65]
                        vB = vall[kt_i][:, (2 * hp + 1) * 65:
                                        (2 * hp + 1) * 65 + 65]
                        nc.tensor.matmul(pyA[:, off:QBS], vA, e2[:, 0:N],
                                         start=(kt_i == 0),
                                         stop=(kt_i == nkt - 1))
                        nc.tensor.matmul(pyB[:, off:QBS], vB,
                                         e2[:, QBS:QBS + N],
                                         start=(kt_i == 0),
                                         stop=(kt_i == nkt - 1))

                    # ---- attention for this head pair ----
                    for qb in range(NQB):
                        q0 = qb * QBS
                        pyA = psyt.tile([65, QBS], F32, name="pyA", tag="ytA")
                        pyB = psyt.tile([65, QBS], F32, name="pyB", tag="ytB")
                        nfull = q0 // 128
                        ndiag = QBS // 128
                        nkt = nfull + ndiag
                        pend = None
                        for kt_i in range(nkt):
                            g = score_group(kt_i, nfull, q0)
                            # background QKV work lands between the score and
                            # the exp-dependent yT so the exp latency is
                            # hidden without blocking the in-order PE stream
                            if bg is not None:
                                next(bg, None)
                            if pend is not None:
                                yt_group(pend, pyA, pyB, nkt)
                            pend = g
                        yt_group(pend, pyA, pyB, nkt)

                        # release the psum accumulators to SBUF immediately,
                        # normalize from the SBUF copies
                        yAsb = normp.tile([65, QBS], F32, name="yAsb",
                                          tag="yAsb")
                        yBsb = normp.tile([65, QBS], F32, name="yBsb",
                                          tag="yBsb")
                        nc.vector.tensor_copy(yAsb[:], pyA[:])
                        nc.vector.tensor_copy(yBsb[:], pyB[:])
                        r2s = normp.tile([1, 2 * QBS], F32, name="r2s",
                                         tag="r2s")
                        nc.vector.tensor_copy(r2s[0:1, 0:QBS], yAsb[64:65, :])
                        nc.vector.tensor_copy(r2s[0:1, QBS:2 * QBS],
                                              yBsb[64:65, :])
                        r2f = normp.tile([1, 2 * QBS], F32, name="r2f",
                                         tag="r2f")
                        nc.vector.reciprocal_approx_fast(r2f[:], r2s[:])
                        # broadcast the reciprocal rows to 128 partitions on
                        # the otherwise-idle GPSIMD engine (replaces two K=1
                        # matmuls + three DVE casts)
                        bc2 = normp.tile([128, 2 * QBS], F32, name="bc2",
                                         tag="bc2")
                        nc.gpsimd.partition_broadcast(bc2[:], r2f[:])
                        qbs = slice(q0, q0 + QBS)
                        nc.vector.tensor_mul(ytall[hp][0:64, qbs],
                                             yAsb[0:64, :], bc2[0:64, 0:QBS])
                        nc.vector.tensor_mul(ytall[hp][64:128, qbs],
                                             yBsb[0:64, :],
                                             bc2[0:64, QBS:2 * QBS])
                        if v_bias:
                            nc.vector.tensor_scalar_add(
                                ytall[hp][0:64, qbs], ytall[hp][0:64, qbs],
                                bv[0:64, hp:hp + 1])
                            nc.vector.tensor_scalar_add(
                                ytall[hp][64:128, qbs], ytall[hp][64:128, qbs],
                                bv[64:128, hp:hp + 1])

                    if bg is not None:
                        for _ in bg:
                            pass
                        cur = nxt

            # ================= phase 3: out projection =================
            with (
                tc.tile_pool(name="wo", bufs=1) as wop,
                tc.tile_pool(name="ost", bufs=4) as ostp,
            ):
                wo = []
                for d_ in range(DC):
                    wod = wop.tile([128, C], BF16, name=f"wo{d_}", tag=f"wo{d_}")
                    nc.sync.dma_start(wod[:], wo_d.ap()[d_])
                    wo.append(wod)
                for t in range(TT):
                    for cs in range(NCS):
                        po = psmm.tile([128, CS], F32, name="po", tag="mm")
                        for d_ in range(DC):
                            nc.tensor.matmul(
                                po[:],
                                ytall[d_][:, t * 128:(t + 1) * 128],
                                wo[d_][:, cs * CS:(cs + 1) * CS],
                                start=(d_ == 0), stop=(d_ == DC - 1),
                            )
                        st = ostp.tile([128, CS], F32, name="st", tag="ost")
                        nc.scalar.copy(st[:], po[:])
                        nc.sync.dma_start(
                            out_d.ap()[t * 128:(t + 1) * 128,
                                       cs * CS:(cs + 1) * CS],
                            st[:])

    nc.compile()
    return nc


# ---------------------------------------------------------------------------
# host-side sharding
# ---------------------------------------------------------------------------

def _planar_perm():
    """Within-head column permutation: even dims -> 0..31, odd -> 32..63."""
    p = np.empty(HEAD_DIM, dtype=np.int64)
    p[:32] = 2 * np.arange(32)
    p[32:] = 2 * np.arange(32) + 1
    return p


def _rope_tables(T):
    theta = 1.0 / (10000.0 ** (np.arange(0, HEAD_DIM, 2, dtype=np.float64)
                               / HEAD_DIM))  # [32]
    idx = np.outer(np.arange(T, dtype=np.float64), theta)  # [T, 32]
    cos = np.cos(idx).astype(np.float32)
    sin = np.sin(idx).astype(np.float32)
    cosf = np.empty((128, T), dtype=np.float32)
    sinf = np.empty((128, T), dtype=np.float32)
    for r in range(128):
        i = r % 32
        lo = ((r // 32) % 2 == 0)
        cosf[r] = cos[:, i]
        sinf[r] = (-sin[:, i]) if lo else sin[:, i]
    # device reads the sin table at the *source* rows of the pair swap
    # (row r holds the value destined for row r^32), so swap 32-row blocks
    sinf = sinf.reshape(4, 32, T)[[1, 0, 3, 2]].reshape(128, T)
    return cosf, sinf


def make_in_maps(x, W_qkv, b_qkv, W_proj, T, C, HLOC, qk_bias, v_bias):
    B = x.shape[0]
    D = HEAD_DIM
    HP = HLOC // 2
    CL = HLOC * D
    KC = C // 128
    DC = CL // 128
    NGRP = (C // D) // HLOC  # head groups

    Wq = np.asarray(W_qkv[:, 0:C], dtype=np.float32)
    Wk = np.asarray(W_qkv[:, C:2 * C], dtype=np.float32)
    Wv = np.asarray(W_qkv[:, 2 * C:3 * C], dtype=np.float32)
    bq = np.asarray(b_qkv[0:C], dtype=np.float32)
    bk = np.asarray(b_qkv[C:2 * C], dtype=np.float32)
    bv_ = np.asarray(b_qkv[2 * C:3 * C], dtype=np.float32)

    perm = _planar_perm()
    cosf, sinf = _rope_tables(T)
    tri = (np.arange(128)[:, None] <= np.arange(128)[None, :])
    onesw = np.zeros((2, 128), dtype=NPBF16)
    onesw[0, 0:64] = 1.0
    onesw[1, 64:128] = 1.0

    def to_bf(a):
        return np.ascontiguousarray(a.astype(NPBF16))

    grp = {}
    for g in range(NGRP):
        cols_qk = np.concatenate(
            [(g * HLOC + h) * D + perm for h in range(HLOC)])
        cols_v = np.concatenate(
            [(g * HLOC + h) * D + np.arange(D) for h in range(HLOC)])
        wq_g = Wq[:, cols_qk]   # [C, CL]
        wk_g = Wk[:, cols_qk]
        wv_g = Wv[:, cols_v]
        wo_g = np.asarray(W_proj[g * CL:(g + 1) * CL, :], dtype=np.float32)

        ent = {
            "wq": to_bf(wq_g.reshape(KC, 128, HP, 128)
                        .transpose(2, 1, 0, 3).reshape(HP, 128, KC * 128)),
            "wk": to_bf(wk_g.reshape(KC, 128, HP, 128)
                        .transpose(2, 1, 0, 3).reshape(HP, 128, KC * 128)),
            "wv": to_bf(wv_g.reshape(KC, 128, CL)),
            "wo": to_bf(wo_g.reshape(DC, 128, C)),
            "cosf": to_bf(cosf),
            "sinf": to_bf(sinf),
            "tri": to_bf(tri.astype(np.float32)),
            "onesw": onesw,
        }
        if qk_bias:
            bqk_t = np.empty((128, 2 * HP), dtype=np.float32)
            bq_g = bq[cols_qk]
            bk_g = bk[cols_qk]
            for hp in range(HP):
                bqk_t[:, hp] = bq_g[hp * 128:(hp + 1) * 128]
                bqk_t[:, HP + hp] = bk_g[hp * 128:(hp + 1) * 128]
            ent["bqk"] = bqk_t
        if v_bias:
            bv_t = np.empty((128, HP), dtype=np.float32)
            bv_g = bv_[cols_v]
            for hp in range(HP):
                bv_t[:, hp] = bv_g[hp * 128:(hp + 1) * 128]
            ent["bv"] = bv_t
        grp[g] = ent

    in_maps = []
    for core in range(B * NGRP):
        b, g = core // NGRP, core % NGRP
        m = dict(grp[g])
        m["xt"] = to_bf(np.asarray(x[b], dtype=np.float32).T)
        in_maps.append(m)
    return in_maps


_CACHE = {}


def _get_graph(T, C, HLOC, qk_bias, v_bias):
    key = (T, C, HLOC, qk_bias, v_bias)
    if key not in _CACHE:
        _CACHE[key] = build_graph(T, C, HLOC, qk_bias, v_bias)
    return _CACHE[key]


def _ensure_ntff_hook():
    """Register the axon NTFF profile hook if the image's antenv lacks it."""
    import sys
    import types
    import antenv
    try:
        from antenv import axon_hooks  # noqa: F401
    except ImportError:
        mod = types.ModuleType("antenv.axon_hooks")
        mod._hook = None

        def set_axon_ntff_profile_hook(h, _m=mod):
            _m._hook = h

        def get_axon_ntff_profile_hook(_m=mod):
            return _m._hook

        mod.set_axon_ntff_profile_hook = set_axon_ntff_profile_hook
        mod.get_axon_ntff_profile_hook = get_axon_ntff_profile_hook
        sys.modules["antenv.axon_hooks"] = mod
        antenv.axon_hooks = mod
    from antenv.axon_hooks import (get_axon_ntff_profile_hook,
                                   set_axon_ntff_profile_hook)
    if get_axon_ntff_profile_hook() is None:
        from trn_agent_boot.trn_boot import _ntff_profile_via_ctypes
        set_axon_ntff_profile_hook(
            _ntff_profile_via_ctypes("/opt/axon/libaxon_pjrt.so"))


def run(inputs, trace=False):
    from concourse.bass_utils import run_bass_kernel_spmd
    if trace:
        try:
            _ensure_ntff_hook()
        except Exception as e:
            print(f"ntff hook setup failed: {e}")
    x = np.asarray(inputs["x"])
    W_qkv = np.asarray(inputs["W_qkv"])
    b_qkv = np.asarray(inputs["b_qkv"])
    W_proj = np.asarray(inputs["W_proj"])
    b_proj = np.asarray(inputs["b_proj"])
    B, T, C = x.shape
    HLOC = HLOC_FULL
    NGRP = (C // HEAD_DIM) // HLOC

    qk_bias = bool(np.any(b_qkv[0:2 * C]))
    v_bias = bool(np.any(b_qkv[2 * C:]))
    nc = _get_graph(T, C, HLOC, qk_bias, v_bias)
    in_maps = make_in_maps(x, W_qkv, b_qkv, W_proj, T, C, HLOC,
                           qk_bias, v_bias)
    res = run_bass_kernel_spmd(nc, in_maps, core_ids=list(range(len(in_maps))),
                               trace=trace)
    out = np.empty((B, T, C), dtype=np.float32)
    for b in range(B):
        acc = None
        for g in range(NGRP):
            part = res.results[b * NGRP + g]["out"]
            acc = part if acc is None else acc + part
        out[b] = acc + b_proj[None, :].astype(np.float32)
    return out, res


def kernel(**inputs):
    out, _ = run(inputs, trace=False)
    return out
